# revision 18
# baseline (speedup 1.0000x reference)
"""GAT-style node attention on 8 TRN2 NeuronCores.

Reference computation (all fp32):
    h = x @ W                                  [N, F]
    e = leaky_relu((h@a1)[:,None] + (h@a2)[None,:], 0.01)
    attention = softmax(where(adj>0, e, -9e15), axis=1)
    out = leaky_relu(attention @ h, 0.01)

Sharding: rows i of attention are split across 8 cores (1024 rows each).
Each core computes the attention block TRANSPOSED (attnT[j, i_local], j on
partitions) so the pm tiles feed the PE directly as lhsT for the
(attention @ h) GEMM -- no on-device transposes anywhere.

Identities used:
    softmax needs no max-subtraction (|z| <~ 25, exp is safe in fp32)
    masked softmax:  p = adj * exp(lrelu(z));  attention = p / rowsum(p)
    rowsum via an appended ones-column in the rhs:  [h | 1]
    exp(lrelu(z)) = max(exp(z), exp(0.01 z))
                  = max(E1[i]*E2[j], E1s[i]*E2s[j])   (separable outer products)
    e1 = x @ (W @ a1), e2 = x @ (W @ a2)  (W@a precomputed on host)

Matmuls run as float32r (full fp32 precision, 1 cycle/row at N>=256).
"""

import sys

for _p in ("/opt/trn_rl_repo",):
    if _p not in sys.path:
        sys.path.insert(0, _p)

from contextlib import ExitStack

import numpy as np

import concourse.bass as bass
import concourse.tile as tile
from concourse import bacc, mybir

dt = mybir.dt
AF = mybir.ActivationFunctionType
ALU = mybir.AluOpType

# Problem sizes (hardcoded per contract)
N = 8192
IN_FT = 512
F = 256
NCORES = 8
SLOPE = 0.01

# Tunables
PM_BUFS = 8        # pm tiles in flight (SBUF)
LV_BUFS = 3        # l/v intermediates
X_BUFS = 4         # xT chunk tiles in flight
ADJ_BUFS = 6       # adj u8 tiles in flight
SCHEME_C_MOD = 5   # jt % SCHEME_C_MOD == SCHEME_C_MOD-1 -> DVE (separable) path
GPS_MASK_MOD = 2   # jt % GPS_MASK_MOD == 0 -> mask multiply on GPSIMD


def build_kernel(n=N, loc=None, elem_dt=dt.float32):
    """Build the SPMD Bass module for one core handling `loc` attention rows."""
    loc = loc if loc is not None else n // NCORES
    kt_n = IN_FT // 128          # k tiles of the x@W contraction
    jt_n = n // 128              # j tiles (softmax/contraction dim)
    it_n = loc // 128            # local i tiles (output rows)
    f32 = dt.float32
    f32r = dt.float32r

    nc = bacc.Bacc("TRN2", target_bir_lowering=False, debug=False)

    xT_d = nc.dram_tensor("xT", [IN_FT, n], f32r, kind="ExternalInput")
    wext_d = nc.dram_tensor("W_ext", [IN_FT, F + 2], f32r, kind="ExternalInput")
    wa1_d = nc.dram_tensor("wa1", [IN_FT, 128], f32r, kind="ExternalInput")
    xtl_d = nc.dram_tensor("xTloc", [IN_FT, loc], f32r, kind="ExternalInput")
    adjT_d = nc.dram_tensor("adjT", [n, loc], dt.uint8, kind="ExternalInput")
    out_d = nc.dram_tensor("out", [loc, F], f32, kind="ExternalOutput")
    scr_d = nc.dram_tensor("scratch", [4, loc], f32)

    with tile.TileContext(nc) as tc, ExitStack() as ctx:
        const_pool = ctx.enter_context(tc.tile_pool(name="const", bufs=1))
        h_pool = ctx.enter_context(tc.tile_pool(name="h", bufs=jt_n))
        pm_pool = ctx.enter_context(tc.tile_pool(name="pm", bufs=PM_BUFS))
        lv_pool = ctx.enter_context(tc.tile_pool(name="lv", bufs=LV_BUFS))
        x_pool = ctx.enter_context(tc.tile_pool(name="xch", bufs=X_BUFS))
        adj_pool = ctx.enter_context(tc.tile_pool(name="adj", bufs=ADJ_BUFS))
        out_pool = ctx.enter_context(tc.tile_pool(name="outp", bufs=2))

        # ---- constants into SBUF ----
        wext_sb = const_pool.tile([128, kt_n, F + 2], f32r)
        nc.sync.dma_start(
            wext_sb[:], wext_d.ap().rearrange("(kt p) f -> p kt f", p=128)
        )
        wa1_sb = const_pool.tile([128, kt_n, 128], f32r)
        nc.sync.dma_start(wa1_sb[:], wa1_d.ap().rearrange("(kt p) f -> p kt f", p=128))
        xtl_sb = const_pool.tile([128, kt_n, loc], f32r)
        nc.sync.dma_start(xtl_sb[:], xtl_d.ap().rearrange("(kt p) i -> p kt i", p=128))

        # ---- phase 0: e1 row vectors ----
        # e1[i] = x[i] @ w1 for local i; rows: [e1; 0.01*e1] via wa1 = [w1, 0.01*w1]
        e1_sb = const_pool.tile([2, loc], f32)
        ch_sz = min(512, loc)
        ch_n = loc // ch_sz
        with tc.tile_pool(name="ps_e1", bufs=2, space="PSUM") as ps_e1_pool:
            for ch in range(ch_n):
                ps = ps_e1_pool.tile([128, ch_sz], f32)
                sl = slice(ch * ch_sz, (ch + 1) * ch_sz)
                for kt in range(kt_n):
                    nc.tensor.matmul(
                        ps[:],
                        wa1_sb[:, kt, :],
                        xtl_sb[:, kt, sl],
                        start=(kt == 0),
                        stop=(kt == kt_n - 1),
                    )
                nc.vector.tensor_copy(e1_sb[:, sl], ps[0:2, :])

        # E rows: [exp(e1); exp(0.01*e1)]
        E_sb = const_pool.tile([2, loc], f32)
        nc.scalar.activation(E_sb[:], e1_sb[:], AF.Exp)

        # broadcast the 4 rows across all 128 partitions (DRAM round-trip)
        nc.sync.dma_start(scr_d[0:2, :], e1_sb[:])
        nc.sync.dma_start(scr_d[2:4, :], E_sb[:])
        e1_rep = const_pool.tile([128, loc], f32)
        e1s_rep = const_pool.tile([128, loc], f32)
        E1_rep = const_pool.tile([128, loc], elem_dt)
        E1s_rep = const_pool.tile([128, loc], elem_dt)
        for r, t in enumerate((e1_rep, e1s_rep, E1_rep, E1s_rep)):
            nc.sync.dma_start(t[:], scr_d[r : r + 1, :].broadcast_to([128, loc]))

        # ---- phase 1: h_ext = x @ W_ext for all j tiles ----
        # h_sb[jt] layout: [0:F]=h, F=ones, F+1=e2, F+2=0.01*e2   (width F+3)
        h_sb = []
        e2pack = const_pool.tile([128, jt_n, 2], f32)
        with tc.tile_pool(name="ps_h", bufs=3, space="PSUM") as ps_h_pool:
            for jt in range(jt_n):
                xch = x_pool.tile([128, kt_n, 128], f32r)
                nc.sync.dma_start(
                    xch[:],
                    xT_d.ap()[:, jt * 128 : (jt + 1) * 128].rearrange(
                        "(kt p) m -> p kt m", p=128
                    ),
                )
                ps = ps_h_pool.tile([128, F + 2], f32)
                for kt in range(kt_n):
                    nc.tensor.matmul(
                        ps[:],
                        xch[:, kt, :],
                        wext_sb[:, kt, :],
                        start=(kt == 0),
                        stop=(kt == kt_n - 1),
                    )
                ht = h_pool.tile([128, F + 2], f32r, tag="h", name=f"h{jt}")
                nc.vector.tensor_copy(ht[:, 0:F], ps[:, 0:F])
                nc.gpsimd.memset(ht[:, F : F + 1].bitcast(f32), 1.0)
                nc.gpsimd.memset(ht[:, F + 1 : F + 2].bitcast(f32), 0.0)
                nc.vector.tensor_copy(e2pack[:, jt, :], ps[:, F : F + 2])
                h_sb.append(ht)

        # E2 columns: exp of [e2, 0.01*e2] per j tile, grouped 8 tiles/op
        E2pack = const_pool.tile([128, jt_n, 2], f32)
        g_sz = min(8, jt_n)
        for g in range(0, jt_n, g_sz):
            nc.scalar.activation(
                E2pack[:, g : g + g_sz, :], e2pack[:, g : g + g_sz, :], AF.Exp
            )

        # ---- phase 2: attention tiles + accumulation ----
        with tc.tile_pool(name="ps_at", bufs=it_n, space="PSUM") as ps_at_pool:
            acc = [
                ps_at_pool.tile([128, F + 2], f32, tag="acc", name=f"acc{it}")
                for it in range(it_n)
            ]

            for jt in range(jt_n):
                pm = pm_pool.tile([128, loc], f32r, tag="pm")
                use_dve = (jt % SCHEME_C_MOD) == SCHEME_C_MOD - 1
                v = lv_pool.tile([128, loc], elem_dt, tag="v")
                if use_dve:
                    # separable path (VectorE): pm = max(E1*E2, E1s*E2s)
                    nc.vector.tensor_scalar(
                        v[:], E1s_rep[:], E2pack[:, jt, 1:2], None, ALU.mult
                    )
                    nc.vector.scalar_tensor_tensor(
                        pm[:], E1_rep[:], E2pack[:, jt, 0:1], v[:], ALU.mult, ALU.max
                    )
                else:
                    # exp path (ScalarE): pm = max(exp(e1+e2), exp(0.01*(e1+e2)))
                    nc.scalar.activation(
                        pm[:], e1_rep[:], AF.Exp, bias=e2pack[:, jt, 0:1]
                    )
                    nc.scalar.activation(
                        v[:], e1s_rep[:], AF.Exp, bias=e2pack[:, jt, 1:2]
                    )
                    nc.vector.tensor_tensor(pm[:], pm[:], v[:], ALU.max)
                adj_t = adj_pool.tile([128, loc], dt.uint8)
                nc.sync.dma_start(adj_t[:], adjT_d.ap()[jt * 128 : (jt + 1) * 128, :])
                if jt % GPS_MASK_MOD == 0:
                    nc.gpsimd.tensor_tensor(pm[:], pm[:], adj_t[:], ALU.mult)
                else:
                    nc.vector.tensor_tensor(pm[:], pm[:], adj_t[:], ALU.mult)

                rhs = h_sb[jt][:, 0 : F + 2]
                for it in range(it_n):
                    nc.tensor.matmul(
                        acc[it][:],
                        pm[:, it * 128 : (it + 1) * 128],
                        rhs,
                        start=(jt == 0),
                        stop=(jt == jt_n - 1),
                    )

            # ---- phase 3: normalize + leaky_relu + store ----
            for it in range(it_n):
                recip = out_pool.tile([128, 1], f32, tag="recip")
                nc.vector.reciprocal(recip[:], acc[it][:, F : F + 1])
                t = out_pool.tile([128, F], f32, tag="t")
                nc.vector.tensor_scalar(t[:], acc[it][:, 0:F], recip[:], None, ALU.mult)
                ot = out_pool.tile([128, F], f32, tag="ot")
                nc.vector.scalar_tensor_tensor(
                    ot[:], t[:], SLOPE, t[:], ALU.mult, ALU.max
                )
                nc.sync.dma_start(out_d.ap()[it * 128 : (it + 1) * 128, :], ot[:])

    nc.compile()
    return nc


def host_prep(x, adj, W, a, n=N, ncores=NCORES):
    """Build per-core input maps from full inputs."""
    loc = n // ncores
    x = np.asarray(x, dtype=np.float32)
    adj = np.asarray(adj)
    W = np.asarray(W, dtype=np.float32)
    a = np.asarray(a, dtype=np.float32)

    a1 = a[:F, 0]
    a2 = a[F:, 0]
    w1 = (W @ a1).astype(np.float32)
    w2 = (W @ a2).astype(np.float32)

    xT = np.ascontiguousarray(x.T)  # [IN_FT, n]
    W_ext = np.concatenate(
        [W, w2[:, None], (SLOPE * w2)[:, None]], axis=1
    ).astype(np.float32)  # [IN_FT, F+2]
    wa1 = np.zeros((IN_FT, 128), dtype=np.float32)  # padded to 128 cols for PE
    wa1[:, 0] = w1
    wa1[:, 1] = SLOPE * w1

    adj_u8 = (adj > 0).astype(np.uint8)

    in_maps = []
    for c in range(ncores):
        rows = slice(c * loc, (c + 1) * loc)
        in_maps.append(
            {
                "xT": xT,
                "W_ext": W_ext,
                "wa1": wa1,
                "xTloc": np.ascontiguousarray(xT[:, rows]),
                "adjT": np.ascontiguousarray(adj_u8[rows, :].T),
            }
        )
    return in_maps


_CACHE = {}


def kernel(x, adj, W, a):
    from concourse.bass_utils import run_bass_kernel_spmd

    key = "full"
    if key not in _CACHE:
        _CACHE[key] = build_kernel()
    nc = _CACHE[key]

    in_maps = host_prep(x, adj, W, a)
    res = run_bass_kernel_spmd(nc, in_maps, list(range(NCORES)))
    out = np.concatenate([res.results[c]["out"] for c in range(NCORES)], axis=0)
    return out.astype(np.float32)


if __name__ == "__main__":
    # smoke test via CoreSim on a reduced size
    from concourse.bass_interp import CoreSim

    rng = np.random.default_rng(0)
    n_s, loc_s = 512, 256
    x = rng.standard_normal((n_s, IN_FT), dtype=np.float32)
    adj = (rng.integers(0, 2, (n_s, n_s))).astype(np.int32)
    W = (rng.standard_normal((IN_FT, F), dtype=np.float32) * (1.414 / np.sqrt(IN_FT))).astype(np.float32)
    a = (rng.standard_normal((2 * F, 1), dtype=np.float32) * 0.1).astype(np.float32)

    # numpy reference for rows [0, loc_s)
    h = x @ W
    e1 = h @ a[:F, 0]
    e2 = h @ a[F:, 0]
    z = e1[:loc_s, None] + e2[None, :]
    e = np.where(z > 0, z, SLOPE * z)
    logits = np.where(adj[:loc_s] > 0, e, -9e15)
    logits -= logits.max(axis=1, keepdims=True)
    p = np.exp(logits)
    att = p / p.sum(axis=1, keepdims=True)
    hp = att @ h
    expected = np.where(hp > 0, hp, SLOPE * hp)

    nc = build_kernel(n=n_s, loc=loc_s)
    maps = host_prep(x, adj, W, a, n=n_s, ncores=n_s // loc_s)
    sim = CoreSim(nc)
    for k, v in maps[0].items():
        sim.tensor(k)[:] = v
    sim.simulate()
    actual = np.array(sim.tensor("out"))
    err = np.abs(actual - expected).max() / (np.abs(expected).max() + 1e-30)
    print("sim rel err:", err)
